# revision 28
# baseline (speedup 1.0000x reference)
"""Bahdanau attention TRN2 Bass kernel (v2).

kernel(**inputs) takes the FULL inputs (as produced by setup_inputs()):
    dec_hidden [32, 1024] f32, enc_outputs [32, 2048, 2048] f32,
    W_s [1024, 1024] f32, W_h [1024, 2048] f32, v [1024] f32
and returns (ctx [32, 2048] f32, attn [32, 2048] f32), matching

    s      = dec_hidden @ W_s.T
    h      = enc_outputs @ W_h.T
    scores = einsum('bld,d->bl', tanh(s[:,None,:] + h), v)
    attn   = softmax(scores, axis=1)
    ctx    = einsum('bl,ble->be', attn, enc_outputs)

Distribution: data-parallel over batch, 4 batch elements per NeuronCore on
8 cores, no collectives.

On-core design (v2 — PE-minimal):
  - enc / W_h / W_s are fed to the device as bf16 (host-side cast; the
    2e-2 tolerance leaves ~40x headroom over bf16 quantization error).
  - encT ([e, l] layout for the h-matmul) is produced by DMA XBAR
    transposes straight from DRAM -- zero PE cycles (the v1 kernel spent
    ~18% of the PE on fp32 transposes).
  - The v-dot uses a column-replicated v as the stationary operand so the
    score PSUM is [128, 512] with identical rows: the flash softmax
    (max / exp / weights) then runs partition-parallel with no broadcast.
  - ctx accumulates on the DVE via tensor_tensor_reduce against the
    encT tiles still resident in SBUF (contraction over l is a free-axis
    reduction in the [e, l] layout), removing the PE ctx matmul and the
    natural-layout enc copy entirely.  enc is read from HBM exactly once.
  - The only PE work left: the irreducible h-matmul (1.048M cycles/core),
    the v-dot (66k), and a few tiny setup transposes.
"""

import json as _json
from contextlib import ExitStack

import numpy as np

_B, _L, _D, _E = 32, 2048, 1024, 2048
_NCORES = 8
_RING_SPLIT = False  # split encT XBAR transposes across both HWDGE rings


# ----------------------------------------------------------------------------
# Workaround: this walrus build rejects instructions carrying more than one
# semaphore wait ("Too many sync wait commands").  Split extra waits onto
# preceding same-engine NoOps at BIR-serialization time.
# ----------------------------------------------------------------------------
_ws_counter = [0]


def _split_instruction_waits(inst, max_waits=1):
    waits = inst.get("sync_info", {}).get("on_wait") or []
    if len(waits) <= max_waits:
        return [inst]
    out = []
    extra = waits[:-max_waits]
    inst["sync_info"]["on_wait"] = waits[-max_waits:]
    for i in range(0, len(extra), max_waits):
        _ws_counter[0] += 1
        out.append({
            "debug": inst.get("debug", 0),
            "engine": inst["engine"],
            "ins": [],
            "name": f"I-ws{_ws_counter[0]}",
            "opcode": "NoOp",
            "outs": [],
            "sync_info": {"on_update": [], "on_wait": extra[i:i + max_waits]},
        })
    out.append(inst)
    return out


def _walk_split(obj):
    if isinstance(obj, dict):
        for key, val in obj.items():
            if key == "instructions" and isinstance(val, list):
                new = []
                for inst in val:
                    if isinstance(inst, dict) and "sync_info" in inst:
                        new.extend(_split_instruction_waits(inst))
                    else:
                        _walk_split(inst)
                        new.append(inst)
                obj[key] = new
            else:
                _walk_split(val)
    elif isinstance(obj, list):
        for item in obj:
            _walk_split(item)


def _install_waitsplit():
    import concourse.bass as bass
    if getattr(bass.Bass, "_waitsplit_installed", False):
        return
    orig = bass.Bass.to_json_bytes

    def to_json_bytes(self, *a, **kw):
        d = _json.loads(orig(self, *a, **kw))
        _walk_split(d)
        return _json.dumps(d).encode()

    bass.Bass.to_json_bytes = to_json_bytes
    bass.Bass._waitsplit_installed = True


# ----------------------------------------------------------------------------
# Kernel builder
# ----------------------------------------------------------------------------

def _build(Bc=4, L=_L, D=_D, E=_E, LCHUNK=512):
    import concourse.bass as bass
    import concourse.mybir as mybir
    import concourse.tile as tile
    from concourse.masks import make_identity

    F32 = mybir.dt.float32
    F32R = mybir.dt.float32r
    BF16 = mybir.dt.bfloat16
    AF = mybir.ActivationFunctionType
    ALU = mybir.AluOpType
    X = mybir.AxisListType.X

    P = 128
    DT, ET, LT = D // P, E // P, L // P
    NCH = L // LCHUNK
    LCT = LCHUNK // P
    assert D % P == 0 and E % P == 0 and L % LCHUNK == 0 and LCHUNK % P == 0

    def r(ap):
        return ap.bitcast(F32R)

    nc = bass.Bass("TRN2", target_bir_lowering=False, debug=False)
    enc = nc.dram_tensor("enc", [Bc, L, E], BF16, kind="ExternalInput").ap()
    dec = nc.dram_tensor("dec", [Bc, D], F32, kind="ExternalInput").ap()
    W_sT = nc.dram_tensor("W_sT", [D, D], BF16, kind="ExternalInput").ap()
    W_hT = nc.dram_tensor("W_hT", [E, D], BF16, kind="ExternalInput").ap()
    v_rep = nc.dram_tensor("v_rep", [DT, P, P], BF16, kind="ExternalInput").ap()
    ctx_o = nc.dram_tensor("ctx", [Bc, E], F32, kind="ExternalOutput").ap()
    attn_o = nc.dram_tensor("attn", [Bc, L], F32, kind="ExternalOutput").ap()

    _cnt = [0]

    with tile.TileContext(nc) as tc:
        with ExitStack() as es:
            const_p = es.enter_context(tc.tile_pool(name="const", bufs=1))
            whT_p = es.enter_context(tc.tile_pool(name="whT", bufs=1))
            sT_p = es.enter_context(tc.tile_pool(name="sT", bufs=1))
            ps_t = es.enter_context(tc.tile_pool(name="ps_t", bufs=1, space="PSUM"))
            ps_h = es.enter_context(tc.tile_pool(name="ps_h", bufs=4, space="PSUM"))
            ps_sc = es.enter_context(tc.tile_pool(name="ps_sc", bufs=2, space="PSUM"))
            ps_row = es.enter_context(tc.tile_pool(name="ps_row", bufs=1, space="PSUM"))
            encT_p = es.enter_context(tc.tile_pool(name="encT", bufs=3))
            t_p = es.enter_context(tc.tile_pool(name="tp", bufs=4))
            w_p = es.enter_context(tc.tile_pool(name="wp", bufs=2))
            rows_p = es.enter_context(tc.tile_pool(name="rows", bufs=2))
            st_p = es.enter_context(tc.tile_pool(name="st", bufs=2))

            ident = const_p.tile([P, P], F32)
            make_identity(nc, ident[:])

            def evac(dst, src):
                """PSUM -> SBUF copy, alternating DVE/ACT."""
                _cnt[0] += 1
                if _cnt[0] % 2 == 0:
                    nc.vector.tensor_copy(out=dst, in_=src)
                else:
                    nc.scalar.copy(out=dst, in_=src)

            # PE warm-up spinner: dead identity MATMULs (transpose-mode
            # does not register as PE-busy for the HAM clock gate) keep the
            # PE busy through the DMA-fed start so HAM releases to 2.4 GHz
            # before the real matmul stream begins.
            for _ in range(32):
                ps = ps_t.tile([P, LCHUNK], F32, tag="ps_t", name="spin")
                nc.tensor.matmul(ps[:, :P], ident[:], ident[:],
                                 start=True, stop=True)

            state = {b: {} for b in range(Bc)}

            whT = [whT_p.tile([P, D], BF16, tag=f"whT{t}", name=f"whT{t}")
                   for t in range(ET)]

            def emit_chunk_load(b, ci, l0, lw, interleave_whT=False):
                """encT via ONE whole-chunk DMA XBAR transpose: a single
                instruction streams ~2.7x faster than 16 per-e-block ones
                (265 vs 98 GB/s measured), and the sync ring stays exclusive
                to XBAR work (two rings transposing concurrently corrupt
                data on HW -- shared XBAR state)."""
                encT = encT_p.tile([P, ET, lw], BF16, tag="encT",
                                   name=f"encT{b}_{ci}")
                nc.sync.dma_start(encT[:], enc[b, l0:l0 + lw, :],
                                  transpose=True)
                if interleave_whT:
                    for t in range(ET):
                        nc.sync.dma_start(whT[t][:],
                                          W_hT[t * P:(t + 1) * P, :])
                state[b][f"encT{ci}"] = encT

            def chunks_of(b):
                """The last batch splits its final 512-chunk in two so the
                exposed flash tail after the last matmul is halved."""
                ch = [(ci * LCHUNK, LCHUNK) for ci in range(NCH)]
                if b == Bc - 1:
                    l0, lwf = ch.pop()
                    ch += [(l0, lwf // 2), (l0 + lwf // 2, lwf // 2)]
                return ch

            # Chunk 0's transpose lands ~8us in; the whT tiles then stream
            # at ~0.65us each, pacing the chunk-0 t-major groups below.
            emit_chunk_load(0, 0, *chunks_of(0)[0], interleave_whT=True)

            # ---- setup: s_b = dec_b @ W_s.T, laid out as sT[J][j', b];
            # ---- v_rep streamed from host (column-replicated v).
            # Small DMAs ride the scalar DGE ring so they don't queue behind
            # the enc transposes; the t-major chunk 0 doesn't need sT until
            # its group-A accumulation closes (~35us), so the slower scalar
            # ring is fine here.
            sT = [sT_p.tile([P, Bc], F32, tag=f"sT{J}", name=f"sT{J}")
                  for J in range(DT)]
            vrep = [sT_p.tile([P, P], BF16, tag=f"vrep{j}", name=f"vrep{j}")
                    for j in range(DT)]
            with tc.tile_pool(name="setup_tmp", bufs=1) as tmp_p, \
                 tc.tile_pool(name="wsT", bufs=1) as wsT_p, \
                 tc.tile_pool(name="decT", bufs=1) as decT_p:
                dec_sb = tmp_p.tile([Bc, D], F32, tag="tmp4k", name="dec_sb")
                nc.scalar.dma_start(dec_sb[:], dec[:, :])
                decT = [decT_p.tile([P, Bc], BF16, tag=f"decT{a}", name=f"decT{a}")
                        for a in range(DT)]
                for a in range(DT):
                    ps = ps_t.tile([P, LCHUNK], F32, tag="ps_t")
                    nc.tensor.transpose(ps[:, :Bc], dec_sb[:, a * P:(a + 1) * P],
                                        ident[0:Bc, 0:Bc])
                    evac(decT[a][:], ps[:, :Bc])
                wst = [wsT_p.tile([P, D], BF16, tag=f"wsT{t}", name=f"wsT{t}")
                       for t in range(DT)]
                for t in range(DT):
                    nc.scalar.dma_start(wst[t][:], W_sT[t * P:(t + 1) * P, :])
                for j in range(DT):
                    nc.scalar.dma_start(vrep[j][:], v_rep[j])
                snat = tmp_p.tile([Bc, D], F32, tag="tmp4k", name="snat")
                SH = (D + 511) // 512
                for h in range(SH):
                    w = min(512, D - 512 * h)
                    ps_sh = ps_row.tile([Bc, 512], F32, tag="ps_row",
                                        name=f"ps_sh{h}")[:, :w]
                    for t in range(DT):
                        nc.tensor.matmul(ps_sh[:], decT[t][:],
                                         wst[t][:, 512 * h:512 * h + w],
                                         start=(t == 0), stop=(t == DT - 1))
                    evac(snat[:, 512 * h:512 * h + w], ps_sh[:])
                for J in range(DT):
                    ps = ps_t.tile([P, LCHUNK], F32, tag="ps_t")
                    nc.tensor.transpose(ps[:, :Bc], snat[:, J * P:(J + 1) * P],
                                        ident[0:Bc, 0:Bc])
                    evac(sT[J][:], ps[:, :Bc])

            def emit_chunk_mm_tmajor(b, ci, l0, lw, JG=4):
                """Batch-0 variant: t-major matmul groups so the PE is never
                head-of-line blocked on a late whT[t] while work on
                already-arrived tiles exists."""
                if ci == 0:
                    state[b]["scores_row"] = rows_p.tile(
                        [1, L], F32, tag="scores_row", name=f"scores_row{b}")
                scores_row = state[b]["scores_row"]
                encT = state[b][f"encT{ci}"]
                psc = ps_sc.tile([P, LCHUNK], F32, tag="ps_sc",
                                 name="psc")[:, :lw]
                for g in range(0, DT, JG):
                    phs = {j: ps_h.tile([P, LCHUNK], F32, tag="ps_h",
                                        name=f"ph{j}")
                           for j in range(g, g + JG)}
                    for t in range(ET):
                        for j in range(g, g + JG):
                            nc.tensor.matmul(phs[j][:, :lw],
                                             whT[t][:, j * P:(j + 1) * P],
                                             encT[:, t, :],
                                             start=(t == 0), stop=(t == ET - 1))
                    tjs = {}
                    for j in range(g, g + JG):
                        tj = t_p.tile([P, LCHUNK], BF16, tag="tj")
                        nc.scalar.activation(tj[:, :lw], phs[j][:, :lw],
                                             AF.Tanh, bias=sT[j][:, b:b + 1])
                        tjs[j] = tj
                    for j in range(g, g + JG):
                        nc.tensor.matmul(psc[:], vrep[j][:],
                                         tjs[j][:, :lw],
                                         start=(j == 0), stop=(j == DT - 1))
                nc.scalar.copy(out=scores_row[:, l0:l0 + lw],
                               in_=psc[0:1, :])
                state[b][f"psc{ci}"] = psc

            def emit_chunk_mm(b, ci, l0, lw):
                if ci == 0:
                    state[b]["scores_row"] = rows_p.tile(
                        [1, L], F32, tag="scores_row", name=f"scores_row{b}")
                scores_row = state[b]["scores_row"]
                encT = state[b][f"encT{ci}"]
                psc = ps_sc.tile([P, LCHUNK], F32, tag="ps_sc",
                                 name="psc")[:, :lw]
                tjs = {}
                for j in range(DT):
                    ph = ps_h.tile([P, LCHUNK], F32, tag="ps_h")
                    for t in range(ET):
                        nc.tensor.matmul(ph[:, :lw],
                                         whT[t][:, j * P:(j + 1) * P],
                                         encT[:, t, :],
                                         start=(t == 0), stop=(t == ET - 1))
                    tj = t_p.tile([P, LCHUNK], BF16, tag="tj")
                    nc.scalar.activation(tj[:, :lw], ph[:, :lw], AF.Tanh,
                                         bias=sT[j][:, b:b + 1])
                    tjs[j] = tj
                    # v-dot for d-block j-1 issues behind j's matmul group, so
                    # the PE never waits on the tanh chain of the same block.
                    if j > 0:
                        nc.tensor.matmul(psc[:], vrep[j - 1][:],
                                         tjs.pop(j - 1)[:, :lw],
                                         start=(j == 1), stop=False)
                nc.tensor.matmul(psc[:], vrep[DT - 1][:],
                                 tjs.pop(DT - 1)[:, :lw],
                                 start=False, stop=True)
                # psc rows are identical; keep row 0 for the attn output.
                nc.scalar.copy(out=scores_row[:, l0:l0 + lw],
                               in_=psc[0:1, :])
                state[b][f"psc{ci}"] = psc

            def emit_flash(b, ci, lw, with_softmax=False):
                """Online-softmax ctx accumulation for chunk ci of batch b.
                All state is [128, 1] / [128, ET] partition-parallel.  For the
                last chunk (with_softmax), the [1, L] attn row normalization
                is emitted right after the running max so its ACT/DMA work
                overlaps the DVE ctx reduction below."""
                psc = state[b].pop(f"psc{ci}")
                encT = state[b].pop(f"encT{ci}")
                par = ci % 2
                mc = st_p.tile([P, 1], F32, tag=f"mc{par}", name=f"mc{par}")
                nc.vector.reduce_max(mc[:], psc[:], axis=X)
                if ci == 0:
                    M = mc
                else:
                    Mold = state[b]["M"]
                    M = st_p.tile([P, 1], F32, tag=f"Mrun{par}", name=f"Mrun{par}")
                    nc.vector.tensor_tensor(M[:], Mold[:], mc[:], ALU.max)
                state[b]["M"] = M
                negM = st_p.tile([P, 1], F32, tag=f"negM{par}", name=f"negM{par}")
                nc.vector.tensor_scalar_mul(negM[:], M[:], -1.0)
                wcb = w_p.tile([P, lw], BF16, tag="wcb", name="wcb")
                zc = st_p.tile([P, 1], F32, tag=f"zc{par}", name=f"zc{par}")
                nc.scalar.activation(wcb[:], psc[:], AF.Exp, bias=negM[:],
                                     accum_out=zc[:])
                if with_softmax:
                    scores_row = state[b]["scores_row"]
                    erow = rows_p.tile([1, L], F32, tag="erow", name=f"erow{b}")
                    zrow = st_p.tile([1, 1], F32, tag="zrow", name="zrow")
                    nc.scalar.activation(erow[:], scores_row[:], AF.Exp,
                                         bias=negM[0:1, :], accum_out=zrow[:])
                    rz = st_p.tile([1, 1], F32, tag="rz", name="rz")
                    nc.vector.reciprocal(rz[:], zrow[:])
                    arow = rows_p.tile([1, L], F32, tag="arow", name=f"arow{b}")
                    nc.scalar.activation(arow[:], erow[:], AF.Copy,
                                         scale=rz[:])
                    nc.sync.dma_start(attn_o[b:b + 1, :], arow[:])
                # per-chunk ctx contribution: cchunk[:, t] = sum_l encT*wcb
                cchunk = st_p.tile([P, ET], F32, tag=f"cchunk{par}",
                                   name=f"cchunk{par}")
                scr = w_p.tile([P, LCHUNK], F32, tag="ttr_scr", name="ttr_scr")
                for t in range(ET):
                    nc.vector.scalar_tensor_tensor(
                        out=scr[:, :lw], in0=encT[:, t, :], scalar=1.0,
                        in1=wcb[:], op0=ALU.mult, op1=ALU.mult,
                        accum_out=cchunk[:, t:t + 1])
                if ci > 0:
                    alpha = st_p.tile([P, 1], F32, tag=f"alpha{par}",
                                      name=f"alpha{par}")
                    nc.scalar.activation(alpha[:], state[b]["Mold"][:], AF.Exp,
                                         bias=negM[:])
                    z_old = state[b]["z"]
                    z = st_p.tile([P, 1], F32, tag=f"z{par}", name=f"z{par}")
                    nc.vector.scalar_tensor_tensor(z[:], z_old[:], alpha[:],
                                                   zc[:], ALU.mult, ALU.add)
                    ctx_old = state[b]["ctx_acc"]
                    ctx_acc = st_p.tile([P, ET], F32, tag=f"ctxn{par}",
                                        name=f"ctxn{par}")
                    nc.vector.scalar_tensor_tensor(ctx_acc[:], ctx_old[:],
                                                   alpha[:], cchunk[:],
                                                   ALU.mult, ALU.add)
                else:
                    z = zc
                    ctx_acc = cchunk
                state[b]["ctx_acc"] = ctx_acc
                state[b]["z"] = z
                state[b]["Mold"] = M

            def emit_ctx_out(b):
                ctx_acc = state[b]["ctx_acc"]
                z = state[b]["z"]
                rz = st_p.tile([P, 1], F32, tag="rz128", name="rz128")
                nc.vector.reciprocal(rz[:], z[:])
                ps = ps_t.tile([P, LCHUNK], F32, tag="ps_t")
                nc.tensor.transpose(ps[0:ET, 0:P], ctx_acc[:], ident[:])
                cout = rows_p.tile([ET, P], F32, tag="cout", name=f"cout{b}")
                # rz rows are identical, so any ET-partition slice of it is
                # the right per-partition scale for the transposed layout.
                nc.scalar.activation(cout[:], ps[0:ET, 0:P], AF.Copy,
                                     scale=rz[0:ET, :])
                nc.sync.dma_start(ctx_o[b].rearrange("(a p) -> a p", a=ET),
                                cout[:])

            # Software pipeline: flash(ci) sits between load(ci+1) and
            # mm(ci+1); batch b+1's first chunk overlaps batch b's softmax
            # tail.  Batch 0 rides the startup DMA feed with t-major groups.
            CH0 = chunks_of(0)
            emit_chunk_mm_tmajor(0, 0, *CH0[0])
            for ci in range(1, len(CH0)):
                emit_chunk_load(0, ci, *CH0[ci])
                emit_flash(0, ci - 1, CH0[ci - 1][1])
                emit_chunk_mm_tmajor(0, ci, *CH0[ci])
            for b in range(Bc):
                CHb = chunks_of(b)
                last = len(CHb) - 1
                if b + 1 < Bc:
                    CHn = chunks_of(b + 1)
                    emit_chunk_load(b + 1, 0, *CHn[0])
                    emit_flash(b, last, CHb[last][1], with_softmax=True)
                    emit_chunk_mm(b + 1, 0, *CHn[0])
                    emit_ctx_out(b)
                    for ci in range(1, len(CHn)):
                        emit_chunk_load(b + 1, ci, *CHn[ci])
                        emit_flash(b + 1, ci - 1, CHn[ci - 1][1])
                        emit_chunk_mm(b + 1, ci, *CHn[ci])
                else:
                    emit_flash(b, last, CHb[last][1], with_softmax=True)
                    emit_ctx_out(b)

    return nc


_cache = {}


def _get_nc():
    if "nc" not in _cache:
        _install_waitsplit()
        _cache["nc"] = _build()
    return _cache["nc"]


def run(inputs, trace=False, **run_kwargs):
    """Run on 8 NeuronCores; returns (ctx, attn, BassKernelResults)."""
    import ml_dtypes
    from concourse.bass_utils import run_bass_kernel_spmd

    nc = _get_nc()
    bf16 = ml_dtypes.bfloat16
    enc = np.ascontiguousarray(
        np.asarray(inputs["enc_outputs"], dtype=np.float32).astype(bf16))
    dec = np.ascontiguousarray(np.asarray(inputs["dec_hidden"], dtype=np.float32))
    W_sT = np.ascontiguousarray(
        np.asarray(inputs["W_s"], dtype=np.float32).T.astype(bf16))
    W_hT = np.ascontiguousarray(
        np.asarray(inputs["W_h"], dtype=np.float32).T.astype(bf16))
    v = np.asarray(inputs["v"], dtype=np.float32)
    v_rep = np.ascontiguousarray(
        np.broadcast_to(v.reshape(_D // 128, 128, 1),
                        (_D // 128, 128, 128)).astype(bf16))
    B = enc.shape[0]
    Bc = B // _NCORES
    in_maps = []
    for i in range(_NCORES):
        in_maps.append({
            "enc": enc[i * Bc:(i + 1) * Bc],
            "dec": dec[i * Bc:(i + 1) * Bc],
            "W_sT": W_sT,
            "W_hT": W_hT,
            "v_rep": v_rep,
        })
    res = run_bass_kernel_spmd(nc, in_maps, core_ids=list(range(_NCORES)),
                               trace=trace, **run_kwargs)
    ctx = np.concatenate([res.results[i]["ctx"] for i in range(_NCORES)], axis=0)
    attn = np.concatenate([res.results[i]["attn"] for i in range(_NCORES)], axis=0)
    return ctx, attn, res


def kernel(**inputs):
    ctx, attn, _ = run(inputs, trace=False)
    return ctx, attn


# revision 31
# speedup vs baseline: 1.0340x; 1.0340x over previous
"""Bahdanau attention TRN2 Bass kernel (v2).

kernel(**inputs) takes the FULL inputs (as produced by setup_inputs()):
    dec_hidden [32, 1024] f32, enc_outputs [32, 2048, 2048] f32,
    W_s [1024, 1024] f32, W_h [1024, 2048] f32, v [1024] f32
and returns (ctx [32, 2048] f32, attn [32, 2048] f32), matching

    s      = dec_hidden @ W_s.T
    h      = enc_outputs @ W_h.T
    scores = einsum('bld,d->bl', tanh(s[:,None,:] + h), v)
    attn   = softmax(scores, axis=1)
    ctx    = einsum('bl,ble->be', attn, enc_outputs)

Distribution: data-parallel over batch, 4 batch elements per NeuronCore on
8 cores, no collectives.

On-core design (v2 — PE-minimal):
  - enc / W_h / W_s are fed to the device as bf16 (host-side cast; the
    2e-2 tolerance leaves ~40x headroom over bf16 quantization error).
  - encT ([e, l] layout for the h-matmul) is produced by DMA XBAR
    transposes straight from DRAM -- zero PE cycles (the v1 kernel spent
    ~18% of the PE on fp32 transposes).
  - The v-dot uses a column-replicated v as the stationary operand so the
    score PSUM is [128, 512] with identical rows: the flash softmax
    (max / exp / weights) then runs partition-parallel with no broadcast.
  - ctx accumulates on the DVE via tensor_tensor_reduce against the
    encT tiles still resident in SBUF (contraction over l is a free-axis
    reduction in the [e, l] layout), removing the PE ctx matmul and the
    natural-layout enc copy entirely.  enc is read from HBM exactly once.
  - The only PE work left: the irreducible h-matmul (1.048M cycles/core),
    the v-dot (66k), and a few tiny setup transposes.
"""

import json as _json
from contextlib import ExitStack

import numpy as np

_B, _L, _D, _E = 32, 2048, 1024, 2048
_NCORES = 8
_RING_SPLIT = False  # split encT XBAR transposes across both HWDGE rings


# ----------------------------------------------------------------------------
# Workaround: this walrus build rejects instructions carrying more than one
# semaphore wait ("Too many sync wait commands").  Split extra waits onto
# preceding same-engine NoOps at BIR-serialization time.
# ----------------------------------------------------------------------------
_ws_counter = [0]


def _split_instruction_waits(inst, max_waits=1):
    waits = inst.get("sync_info", {}).get("on_wait") or []
    if len(waits) <= max_waits:
        return [inst]
    out = []
    extra = waits[:-max_waits]
    inst["sync_info"]["on_wait"] = waits[-max_waits:]
    for i in range(0, len(extra), max_waits):
        _ws_counter[0] += 1
        out.append({
            "debug": inst.get("debug", 0),
            "engine": inst["engine"],
            "ins": [],
            "name": f"I-ws{_ws_counter[0]}",
            "opcode": "NoOp",
            "outs": [],
            "sync_info": {"on_update": [], "on_wait": extra[i:i + max_waits]},
        })
    out.append(inst)
    return out


def _walk_split(obj):
    if isinstance(obj, dict):
        for key, val in obj.items():
            if key == "instructions" and isinstance(val, list):
                new = []
                for inst in val:
                    if isinstance(inst, dict) and "sync_info" in inst:
                        new.extend(_split_instruction_waits(inst))
                    else:
                        _walk_split(inst)
                        new.append(inst)
                obj[key] = new
            else:
                _walk_split(val)
    elif isinstance(obj, list):
        for item in obj:
            _walk_split(item)


def _install_waitsplit():
    import concourse.bass as bass
    if getattr(bass.Bass, "_waitsplit_installed", False):
        return
    orig = bass.Bass.to_json_bytes

    def to_json_bytes(self, *a, **kw):
        d = _json.loads(orig(self, *a, **kw))
        _walk_split(d)
        return _json.dumps(d).encode()

    bass.Bass.to_json_bytes = to_json_bytes
    bass.Bass._waitsplit_installed = True


# ----------------------------------------------------------------------------
# Kernel builder
# ----------------------------------------------------------------------------

def _build(Bc=4, L=_L, D=_D, E=_E, LCHUNK=512):
    import concourse.bass as bass
    import concourse.mybir as mybir
    import concourse.tile as tile
    from concourse.masks import make_identity

    F32 = mybir.dt.float32
    F32R = mybir.dt.float32r
    BF16 = mybir.dt.bfloat16
    AF = mybir.ActivationFunctionType
    ALU = mybir.AluOpType
    X = mybir.AxisListType.X

    P = 128
    DT, ET, LT = D // P, E // P, L // P
    NCH = L // LCHUNK
    LCT = LCHUNK // P
    assert D % P == 0 and E % P == 0 and L % LCHUNK == 0 and LCHUNK % P == 0

    def r(ap):
        return ap.bitcast(F32R)

    nc = bass.Bass("TRN2", target_bir_lowering=False, debug=False)
    enc = nc.dram_tensor("enc", [Bc, L, E], BF16, kind="ExternalInput").ap()
    dec = nc.dram_tensor("dec", [Bc, D], F32, kind="ExternalInput").ap()
    W_sT = nc.dram_tensor("W_sT", [D, D], BF16, kind="ExternalInput").ap()
    W_hT = nc.dram_tensor("W_hT", [E, D], BF16, kind="ExternalInput").ap()
    v_rep = nc.dram_tensor("v_rep", [DT, P, P], BF16, kind="ExternalInput").ap()
    nsmax = nc.dram_tensor("nsmax", [P, 1], F32, kind="ExternalInput").ap()
    ctx_o = nc.dram_tensor("ctx", [Bc, E], F32, kind="ExternalOutput").ap()
    attn_o = nc.dram_tensor("attn", [Bc, L], F32, kind="ExternalOutput").ap()

    _cnt = [0]

    with tile.TileContext(nc) as tc:
        with ExitStack() as es:
            const_p = es.enter_context(tc.tile_pool(name="const", bufs=1))
            whT_p = es.enter_context(tc.tile_pool(name="whT", bufs=1))
            sT_p = es.enter_context(tc.tile_pool(name="sT", bufs=1))
            ps_t = es.enter_context(tc.tile_pool(name="ps_t", bufs=1, space="PSUM"))
            ps_h = es.enter_context(tc.tile_pool(name="ps_h", bufs=4, space="PSUM"))
            ps_sc = es.enter_context(tc.tile_pool(name="ps_sc", bufs=2, space="PSUM"))
            ps_row = es.enter_context(tc.tile_pool(name="ps_row", bufs=1, space="PSUM"))
            encT_p = es.enter_context(tc.tile_pool(name="encT", bufs=3))
            t_p = es.enter_context(tc.tile_pool(name="tp", bufs=4))
            w_p = es.enter_context(tc.tile_pool(name="wp", bufs=2))
            rows_p = es.enter_context(tc.tile_pool(name="rows", bufs=2))
            st_p = es.enter_context(tc.tile_pool(name="st", bufs=2))

            ident = const_p.tile([P, P], F32)
            make_identity(nc, ident[:])

            def evac(dst, src):
                """PSUM -> SBUF copy, alternating DVE/ACT."""
                _cnt[0] += 1
                if _cnt[0] % 2 == 0:
                    nc.vector.tensor_copy(out=dst, in_=src)
                else:
                    nc.scalar.copy(out=dst, in_=src)

            # PE warm-up spinner: dead identity MATMULs (transpose-mode
            # does not register as PE-busy for the HAM clock gate) keep the
            # PE busy through the DMA-fed start so HAM releases to 2.4 GHz
            # before the real matmul stream begins.
            for _ in range(32):
                ps = ps_t.tile([P, LCHUNK], F32, tag="ps_t", name="spin")
                nc.tensor.matmul(ps[:, :P], ident[:], ident[:],
                                 start=True, stop=True)

            state = {b: {} for b in range(Bc)}

            whT = [whT_p.tile([P, D], BF16, tag=f"whT{t}", name=f"whT{t}")
                   for t in range(ET)]

            def emit_chunk_load(b, ci, l0, lw, interleave_whT=False):
                """encT via ONE whole-chunk DMA XBAR transpose: a single
                instruction streams ~2.7x faster than 16 per-e-block ones
                (265 vs 98 GB/s measured), and the sync ring stays exclusive
                to XBAR work (two rings transposing concurrently corrupt
                data on HW -- shared XBAR state)."""
                encT = encT_p.tile([P, ET, lw], BF16, tag="encT",
                                   name=f"encT{b}_{ci}")
                nc.sync.dma_start(encT[:], enc[b, l0:l0 + lw, :],
                                  transpose=True)
                if interleave_whT:
                    for t in range(ET):
                        nc.sync.dma_start(whT[t][:],
                                          W_hT[t * P:(t + 1) * P, :])
                state[b][f"encT{ci}"] = encT

            def chunks_of(b):
                """The last batch splits its final 512-chunk in two so the
                exposed flash tail after the last matmul is halved."""
                ch = [(ci * LCHUNK, LCHUNK) for ci in range(NCH)]
                if b == Bc - 1:
                    l0, lwf = ch.pop()
                    ch += [(l0, lwf // 2), (l0 + lwf // 2, lwf // 2)]
                return ch

            # Chunk 0's transpose lands ~8us in; the whT tiles then stream
            # at ~0.65us each, pacing the chunk-0 t-major groups below.
            emit_chunk_load(0, 0, *chunks_of(0)[0], interleave_whT=True)

            # ---- setup: s_b = dec_b @ W_s.T, laid out as sT[J][j', b];
            # ---- v_rep streamed from host (column-replicated v).
            # Small DMAs ride the scalar DGE ring so they don't queue behind
            # the enc transposes; the t-major chunk 0 doesn't need sT until
            # its group-A accumulation closes (~35us), so the slower scalar
            # ring is fine here.
            sT = [sT_p.tile([P, Bc], F32, tag=f"sT{J}", name=f"sT{J}")
                  for J in range(DT)]
            vrep = [sT_p.tile([P, P], BF16, tag=f"vrep{j}", name=f"vrep{j}")
                    for j in range(DT)]
            # -SMAX = -sum|v| >= -max-score bound: softmax is shift-invariant,
            # and exp(s - SMAX) in [e^-2|v|1, 1] stays comfortably in range,
            # so NO running max / rescale machinery is needed anywhere.
            nsm = sT_p.tile([P, 1], F32, tag="nsm", name="nsm")
            nc.scalar.dma_start(nsm[:], nsmax[:])
            with tc.tile_pool(name="setup_tmp", bufs=1) as tmp_p, \
                 tc.tile_pool(name="wsT", bufs=1) as wsT_p, \
                 tc.tile_pool(name="decT", bufs=1) as decT_p:
                dec_sb = tmp_p.tile([Bc, D], F32, tag="tmp4k", name="dec_sb")
                nc.scalar.dma_start(dec_sb[:], dec[:, :])
                decT = [decT_p.tile([P, Bc], BF16, tag=f"decT{a}", name=f"decT{a}")
                        for a in range(DT)]
                for a in range(DT):
                    ps = ps_t.tile([P, LCHUNK], F32, tag="ps_t")
                    nc.tensor.transpose(ps[:, :Bc], dec_sb[:, a * P:(a + 1) * P],
                                        ident[0:Bc, 0:Bc])
                    evac(decT[a][:], ps[:, :Bc])
                wst = [wsT_p.tile([P, D], BF16, tag=f"wsT{t}", name=f"wsT{t}")
                       for t in range(DT)]
                for t in range(DT):
                    nc.scalar.dma_start(wst[t][:], W_sT[t * P:(t + 1) * P, :])
                for j in range(DT):
                    nc.scalar.dma_start(vrep[j][:], v_rep[j])
                snat = tmp_p.tile([Bc, D], F32, tag="tmp4k", name="snat")
                SH = (D + 511) // 512
                for h in range(SH):
                    w = min(512, D - 512 * h)
                    ps_sh = ps_row.tile([Bc, 512], F32, tag="ps_row",
                                        name=f"ps_sh{h}")[:, :w]
                    for t in range(DT):
                        nc.tensor.matmul(ps_sh[:], decT[t][:],
                                         wst[t][:, 512 * h:512 * h + w],
                                         start=(t == 0), stop=(t == DT - 1))
                    evac(snat[:, 512 * h:512 * h + w], ps_sh[:])
                for J in range(DT):
                    ps = ps_t.tile([P, LCHUNK], F32, tag="ps_t")
                    nc.tensor.transpose(ps[:, :Bc], snat[:, J * P:(J + 1) * P],
                                        ident[0:Bc, 0:Bc])
                    evac(sT[J][:], ps[:, :Bc])

            def emit_chunk_mm_tmajor(b, ci, l0, lw, JG=4):
                """Batch-0 variant: t-major matmul groups so the PE is never
                head-of-line blocked on a late whT[t] while work on
                already-arrived tiles exists."""
                if ci == 0:
                    state[b]["erow"] = rows_p.tile(
                        [1, L], F32, tag="erow", name=f"erow{b}")
                erow = state[b]["erow"]
                encT = state[b][f"encT{ci}"]
                psc = ps_sc.tile([P, LCHUNK], F32, tag="ps_sc",
                                 name="psc")[:, :lw]
                for g in range(0, DT, JG):
                    phs = {j: ps_h.tile([P, LCHUNK], F32, tag="ps_h",
                                        name=f"ph{j}")
                           for j in range(g, g + JG)}
                    for t in range(ET):
                        for j in range(g, g + JG):
                            nc.tensor.matmul(phs[j][:, :lw],
                                             whT[t][:, j * P:(j + 1) * P],
                                             encT[:, t, :],
                                             start=(t == 0), stop=(t == ET - 1))
                    tjs = {}
                    for j in range(g, g + JG):
                        tj = t_p.tile([P, LCHUNK], BF16, tag="tj")
                        nc.scalar.activation(tj[:, :lw], phs[j][:, :lw],
                                             AF.Tanh, bias=sT[j][:, b:b + 1])
                        tjs[j] = tj
                    for j in range(g, g + JG):
                        nc.tensor.matmul(psc[:], vrep[j][:],
                                         tjs[j][:, :lw],
                                         start=(j == 0), stop=(j == DT - 1))
                # unnormalized attn row for this chunk (psc rows identical)
                nc.scalar.activation(erow[:, l0:l0 + lw], psc[0:1, :],
                                     AF.Exp, bias=nsm[0:1, :])
                state[b][f"psc{ci}"] = psc

            def emit_chunk_mm(b, ci, l0, lw):
                if ci == 0:
                    state[b]["erow"] = rows_p.tile(
                        [1, L], F32, tag="erow", name=f"erow{b}")
                erow = state[b]["erow"]
                encT = state[b][f"encT{ci}"]
                psc = ps_sc.tile([P, LCHUNK], F32, tag="ps_sc",
                                 name="psc")[:, :lw]
                tjs = {}
                for j in range(DT):
                    ph = ps_h.tile([P, LCHUNK], F32, tag="ps_h")
                    for t in range(ET):
                        nc.tensor.matmul(ph[:, :lw],
                                         whT[t][:, j * P:(j + 1) * P],
                                         encT[:, t, :],
                                         start=(t == 0), stop=(t == ET - 1))
                    tj = t_p.tile([P, LCHUNK], BF16, tag="tj")
                    nc.scalar.activation(tj[:, :lw], ph[:, :lw], AF.Tanh,
                                         bias=sT[j][:, b:b + 1])
                    tjs[j] = tj
                    # v-dot for d-block j-1 issues behind j's matmul group, so
                    # the PE never waits on the tanh chain of the same block.
                    if j > 0:
                        nc.tensor.matmul(psc[:], vrep[j - 1][:],
                                         tjs.pop(j - 1)[:, :lw],
                                         start=(j == 1), stop=False)
                nc.tensor.matmul(psc[:], vrep[DT - 1][:],
                                 tjs.pop(DT - 1)[:, :lw],
                                 start=False, stop=True)
                # unnormalized attn row for this chunk (psc rows identical)
                nc.scalar.activation(erow[:, l0:l0 + lw], psc[0:1, :],
                                     AF.Exp, bias=nsm[0:1, :])
                state[b][f"psc{ci}"] = psc

            def emit_flash(b, ci, lw):
                """ctx accumulation for chunk ci of batch b, all
                [128, 1] / [128, ET] partition-parallel.  Thanks to the
                fixed -SMAX bias there is no running max: weights are
                exp(s - SMAX), summed and reduced directly."""
                psc = state[b].pop(f"psc{ci}")
                encT = state[b].pop(f"encT{ci}")
                par = ci % 2
                wcb = w_p.tile([P, lw], BF16, tag="wcb", name="wcb")
                zc = st_p.tile([P, 1], F32, tag=f"zc{par}", name=f"zc{par}")
                nc.scalar.activation(wcb[:], psc[:], AF.Exp, bias=nsm[:],
                                     accum_out=zc[:])
                # per-chunk ctx contribution: cchunk[:, t] = sum_l encT*wcb
                cchunk = st_p.tile([P, ET], F32, tag=f"cchunk{par}",
                                   name=f"cchunk{par}")
                scr = w_p.tile([P, LCHUNK], F32, tag="ttr_scr", name="ttr_scr")
                for t in range(ET):
                    nc.vector.scalar_tensor_tensor(
                        out=scr[:, :lw], in0=encT[:, t, :], scalar=1.0,
                        in1=wcb[:], op0=ALU.mult, op1=ALU.mult,
                        accum_out=cchunk[:, t:t + 1])
                if ci > 0:
                    z_old = state[b]["z"]
                    z = st_p.tile([P, 1], F32, tag=f"z{par}", name=f"z{par}")
                    nc.vector.tensor_add(out=z[:], in0=z_old[:], in1=zc[:])
                    ctx_old = state[b]["ctx_acc"]
                    ctx_acc = st_p.tile([P, ET], F32, tag=f"ctxn{par}",
                                        name=f"ctxn{par}")
                    nc.vector.tensor_add(out=ctx_acc[:], in0=ctx_old[:],
                                         in1=cchunk[:])
                else:
                    z = zc
                    ctx_acc = cchunk
                state[b]["ctx_acc"] = ctx_acc
                state[b]["z"] = z

            def emit_ctx_out(b):
                ctx_acc = state[b]["ctx_acc"]
                z = state[b]["z"]
                rz = st_p.tile([P, 1], F32, tag="rz128", name="rz128")
                nc.vector.reciprocal(rz[:], z[:])
                ps = ps_t.tile([P, LCHUNK], F32, tag="ps_t")
                nc.tensor.transpose(ps[0:ET, 0:P], ctx_acc[:], ident[:])
                cout = rows_p.tile([ET, P], F32, tag="cout", name=f"cout{b}")
                # rz rows are identical, so any slice of it is the right
                # per-partition scale in either layout.
                nc.scalar.activation(cout[:], ps[0:ET, 0:P], AF.Copy,
                                     scale=rz[0:ET, :])
                nc.sync.dma_start(ctx_o[b].rearrange("(a p) -> a p", a=ET),
                                  cout[:])
                erow = state[b]["erow"]
                arow = rows_p.tile([1, L], F32, tag="arow", name=f"arow{b}")
                nc.scalar.activation(arow[:], erow[:], AF.Copy,
                                     scale=rz[0:1, :])
                nc.sync.dma_start(attn_o[b:b + 1, :], arow[:])

            # Software pipeline: flash(ci) sits between load(ci+1) and
            # mm(ci+1); batch b+1's first chunk overlaps batch b's softmax
            # tail.  Batch 0 rides the startup DMA feed with t-major groups.
            CH0 = chunks_of(0)
            emit_chunk_mm_tmajor(0, 0, *CH0[0])
            for ci in range(1, len(CH0)):
                emit_chunk_load(0, ci, *CH0[ci])
                emit_flash(0, ci - 1, CH0[ci - 1][1])
                emit_chunk_mm_tmajor(0, ci, *CH0[ci])
            for b in range(Bc):
                CHb = chunks_of(b)
                last = len(CHb) - 1
                if b + 1 < Bc:
                    CHn = chunks_of(b + 1)
                    emit_chunk_load(b + 1, 0, *CHn[0])
                    emit_flash(b, last, CHb[last][1])
                    emit_chunk_mm(b + 1, 0, *CHn[0])
                    emit_ctx_out(b)
                    for ci in range(1, len(CHn)):
                        emit_chunk_load(b + 1, ci, *CHn[ci])
                        emit_flash(b + 1, ci - 1, CHn[ci - 1][1])
                        emit_chunk_mm(b + 1, ci, *CHn[ci])
                else:
                    emit_flash(b, last, CHb[last][1])
                    emit_ctx_out(b)

    return nc


_cache = {}


def _get_nc():
    if "nc" not in _cache:
        _install_waitsplit()
        _cache["nc"] = _build()
    return _cache["nc"]


def run(inputs, trace=False, **run_kwargs):
    """Run on 8 NeuronCores; returns (ctx, attn, BassKernelResults)."""
    import ml_dtypes
    from concourse.bass_utils import run_bass_kernel_spmd

    nc = _get_nc()
    bf16 = ml_dtypes.bfloat16
    enc = np.ascontiguousarray(
        np.asarray(inputs["enc_outputs"], dtype=np.float32).astype(bf16))
    dec = np.ascontiguousarray(np.asarray(inputs["dec_hidden"], dtype=np.float32))
    W_sT = np.ascontiguousarray(
        np.asarray(inputs["W_s"], dtype=np.float32).T.astype(bf16))
    W_hT = np.ascontiguousarray(
        np.asarray(inputs["W_h"], dtype=np.float32).T.astype(bf16))
    v = np.asarray(inputs["v"], dtype=np.float32)
    v_rep = np.ascontiguousarray(
        np.broadcast_to(v.reshape(_D // 128, 128, 1),
                        (_D // 128, 128, 128)).astype(bf16))
    nsmax = np.full((128, 1), -np.abs(v).sum(), dtype=np.float32)
    B = enc.shape[0]
    Bc = B // _NCORES
    in_maps = []
    for i in range(_NCORES):
        in_maps.append({
            "enc": enc[i * Bc:(i + 1) * Bc],
            "dec": dec[i * Bc:(i + 1) * Bc],
            "W_sT": W_sT,
            "W_hT": W_hT,
            "v_rep": v_rep,
            "nsmax": nsmax,
        })
    res = run_bass_kernel_spmd(nc, in_maps, core_ids=list(range(_NCORES)),
                               trace=trace, **run_kwargs)
    ctx = np.concatenate([res.results[i]["ctx"] for i in range(_NCORES)], axis=0)
    attn = np.concatenate([res.results[i]["attn"] for i in range(_NCORES)], axis=0)
    return ctx, attn, res


def kernel(**inputs):
    ctx, attn, _ = run(inputs, trace=False)
    return ctx, attn


# revision 33
# speedup vs baseline: 1.0377x; 1.0035x over previous
"""Bahdanau attention TRN2 Bass kernel (v2).

kernel(**inputs) takes the FULL inputs (as produced by setup_inputs()):
    dec_hidden [32, 1024] f32, enc_outputs [32, 2048, 2048] f32,
    W_s [1024, 1024] f32, W_h [1024, 2048] f32, v [1024] f32
and returns (ctx [32, 2048] f32, attn [32, 2048] f32), matching

    s      = dec_hidden @ W_s.T
    h      = enc_outputs @ W_h.T
    scores = einsum('bld,d->bl', tanh(s[:,None,:] + h), v)
    attn   = softmax(scores, axis=1)
    ctx    = einsum('bl,ble->be', attn, enc_outputs)

Distribution: data-parallel over batch, 4 batch elements per NeuronCore on
8 cores, no collectives.

On-core design (PE-minimal; 529us vs the 826us v1 baseline):
  - enc / W_h / W_s / v are fed to the device as bf16 (host-side cast;
    the 2e-2 tolerance leaves ~5x headroom over bf16 quantization).
  - encT ([e, l] layout for the h-matmul) is produced by ONE whole-chunk
    DMA XBAR transpose per 512-l chunk, straight from DRAM: zero PE
    cycles, and a single instruction streams 2.7x faster than
    per-e-block transposes (265 vs 98 GB/s measured).  All transposes
    stay on the sync ring -- concurrent XBAR use from two DGE rings
    corrupts data on HW.
  - The v-dot uses a column-replicated v as the stationary operand so
    the score PSUM is [128, 512] with identical rows: softmax weights
    come out partition-parallel with no broadcast step.
  - Softmax uses a fixed analytic bound SMAX = sum|v| >= max score
    (softmax is shift-invariant; exp(s-SMAX) >= e^-2|v|1 cannot
    under/overflow), so there is NO running max, rescale, or reduce_max
    anywhere -- the flash "online" machinery reduces to exp + add.
  - ctx accumulates on the DVE via scalar_tensor_tensor free-axis
    reductions against the encT tiles still resident in SBUF, removing
    the PE ctx matmul and the natural-layout enc copy entirely.  enc is
    read from HBM exactly once.
  - PE work: the irreducible h-matmul (1.048M cycles/core), the v-dot
    (66k), tiny setup transposes.  Measured 94% PE occupancy.
  - Startup: a dead-matmul spinner defeats the HAM cold clock (1.2 GHz
    default; transpose-mode does not register as PE-busy), weights ride
    the scalar DGE ring, and batch 0 issues t-major matmul groups so the
    PE is never head-of-line blocked on a late weight tile.  The last
    batch splits its final chunk in two to halve the exposed flash tail.
"""

import json as _json
from contextlib import ExitStack

import numpy as np

_B, _L, _D, _E = 32, 2048, 1024, 2048
_NCORES = 8
_RING_SPLIT = False  # split encT XBAR transposes across both HWDGE rings


# ----------------------------------------------------------------------------
# Workaround: this walrus build rejects instructions carrying more than one
# semaphore wait ("Too many sync wait commands").  Split extra waits onto
# preceding same-engine NoOps at BIR-serialization time.
# ----------------------------------------------------------------------------
_ws_counter = [0]


def _split_instruction_waits(inst, max_waits=1):
    waits = inst.get("sync_info", {}).get("on_wait") or []
    if len(waits) <= max_waits:
        return [inst]
    out = []
    extra = waits[:-max_waits]
    inst["sync_info"]["on_wait"] = waits[-max_waits:]
    for i in range(0, len(extra), max_waits):
        _ws_counter[0] += 1
        out.append({
            "debug": inst.get("debug", 0),
            "engine": inst["engine"],
            "ins": [],
            "name": f"I-ws{_ws_counter[0]}",
            "opcode": "NoOp",
            "outs": [],
            "sync_info": {"on_update": [], "on_wait": extra[i:i + max_waits]},
        })
    out.append(inst)
    return out


def _walk_split(obj):
    if isinstance(obj, dict):
        for key, val in obj.items():
            if key == "instructions" and isinstance(val, list):
                new = []
                for inst in val:
                    if isinstance(inst, dict) and "sync_info" in inst:
                        new.extend(_split_instruction_waits(inst))
                    else:
                        _walk_split(inst)
                        new.append(inst)
                obj[key] = new
            else:
                _walk_split(val)
    elif isinstance(obj, list):
        for item in obj:
            _walk_split(item)


def _install_waitsplit():
    import concourse.bass as bass
    if getattr(bass.Bass, "_waitsplit_installed", False):
        return
    orig = bass.Bass.to_json_bytes

    def to_json_bytes(self, *a, **kw):
        d = _json.loads(orig(self, *a, **kw))
        _walk_split(d)
        return _json.dumps(d).encode()

    bass.Bass.to_json_bytes = to_json_bytes
    bass.Bass._waitsplit_installed = True


# ----------------------------------------------------------------------------
# Kernel builder
# ----------------------------------------------------------------------------

def _build(Bc=4, L=_L, D=_D, E=_E, LCHUNK=512):
    import concourse.bass as bass
    import concourse.mybir as mybir
    import concourse.tile as tile
    from concourse.masks import make_identity

    F32 = mybir.dt.float32
    F32R = mybir.dt.float32r
    BF16 = mybir.dt.bfloat16
    AF = mybir.ActivationFunctionType
    ALU = mybir.AluOpType
    X = mybir.AxisListType.X

    P = 128
    DT, ET, LT = D // P, E // P, L // P
    NCH = L // LCHUNK
    LCT = LCHUNK // P
    assert D % P == 0 and E % P == 0 and L % LCHUNK == 0 and LCHUNK % P == 0

    def r(ap):
        return ap.bitcast(F32R)

    nc = bass.Bass("TRN2", target_bir_lowering=False, debug=False)
    enc = nc.dram_tensor("enc", [Bc, L, E], BF16, kind="ExternalInput").ap()
    dec = nc.dram_tensor("dec", [Bc, D], F32, kind="ExternalInput").ap()
    W_sT = nc.dram_tensor("W_sT", [D, D], BF16, kind="ExternalInput").ap()
    W_hT = nc.dram_tensor("W_hT", [E, D], BF16, kind="ExternalInput").ap()
    v_rep = nc.dram_tensor("v_rep", [DT, P, P], BF16, kind="ExternalInput").ap()
    nsmax = nc.dram_tensor("nsmax", [P, 1], F32, kind="ExternalInput").ap()
    ctx_o = nc.dram_tensor("ctx", [Bc, E], F32, kind="ExternalOutput").ap()
    attn_o = nc.dram_tensor("attn", [Bc, L], F32, kind="ExternalOutput").ap()

    _cnt = [0]

    with tile.TileContext(nc) as tc:
        with ExitStack() as es:
            const_p = es.enter_context(tc.tile_pool(name="const", bufs=1))
            whT_p = es.enter_context(tc.tile_pool(name="whT", bufs=1))
            sT_p = es.enter_context(tc.tile_pool(name="sT", bufs=1))
            ps_t = es.enter_context(tc.tile_pool(name="ps_t", bufs=1, space="PSUM"))
            ps_h = es.enter_context(tc.tile_pool(name="ps_h", bufs=4, space="PSUM"))
            ps_sc = es.enter_context(tc.tile_pool(name="ps_sc", bufs=2, space="PSUM"))
            ps_row = es.enter_context(tc.tile_pool(name="ps_row", bufs=1, space="PSUM"))
            encT_p = es.enter_context(tc.tile_pool(name="encT", bufs=3))
            t_p = es.enter_context(tc.tile_pool(name="tp", bufs=4))
            w_p = es.enter_context(tc.tile_pool(name="wp", bufs=2))
            rows_p = es.enter_context(tc.tile_pool(name="rows", bufs=2))
            st_p = es.enter_context(tc.tile_pool(name="st", bufs=2))

            ident = const_p.tile([P, P], F32)
            make_identity(nc, ident[:])

            def evac(dst, src):
                """PSUM -> SBUF copy, alternating DVE/ACT."""
                _cnt[0] += 1
                if _cnt[0] % 2 == 0:
                    nc.vector.tensor_copy(out=dst, in_=src)
                else:
                    nc.scalar.copy(out=dst, in_=src)

            # PE warm-up spinner: dead identity MATMULs (transpose-mode
            # does not register as PE-busy for the HAM clock gate) keep the
            # PE busy through the DMA-fed start so HAM releases to 2.4 GHz
            # before the real matmul stream begins.
            for _ in range(32):
                ps = ps_t.tile([P, LCHUNK], F32, tag="ps_t", name="spin")
                nc.tensor.matmul(ps[:, :P], ident[:], ident[:],
                                 start=True, stop=True)

            state = {b: {} for b in range(Bc)}

            whT = [whT_p.tile([P, D], BF16, tag=f"whT{t}", name=f"whT{t}")
                   for t in range(ET)]

            def emit_chunk_load(b, ci, l0, lw, interleave_whT=False):
                """encT via ONE whole-chunk DMA XBAR transpose: a single
                instruction streams ~2.7x faster than 16 per-e-block ones
                (265 vs 98 GB/s measured), and the sync ring stays exclusive
                to XBAR work (two rings transposing concurrently corrupt
                data on HW -- shared XBAR state)."""
                encT = encT_p.tile([P, ET, lw], BF16, tag="encT",
                                   name=f"encT{b}_{ci}")
                nc.sync.dma_start(encT[:], enc[b, l0:l0 + lw, :],
                                  transpose=True)
                if interleave_whT:
                    for t in range(ET):
                        nc.sync.dma_start(whT[t][:],
                                          W_hT[t * P:(t + 1) * P, :])
                state[b][f"encT{ci}"] = encT

            def chunks_of(b):
                """The last batch splits its final 512-chunk in two so the
                exposed flash tail after the last matmul is halved."""
                ch = [(ci * LCHUNK, LCHUNK) for ci in range(NCH)]
                if b == Bc - 1:
                    l0, lwf = ch.pop()
                    h = lwf // 2
                    q = lwf // 4
                    ch += [(l0, h), (l0 + h, q), (l0 + h + q, q)]
                return ch

            # Chunk 0's transpose lands ~8us in; the whT tiles then stream
            # at ~0.65us each, pacing the chunk-0 t-major groups below.
            emit_chunk_load(0, 0, *chunks_of(0)[0], interleave_whT=True)

            # ---- setup: s_b = dec_b @ W_s.T, laid out as sT[J][j', b];
            # ---- v_rep streamed from host (column-replicated v).
            # Small DMAs ride the scalar DGE ring so they don't queue behind
            # the enc transposes; the t-major chunk 0 doesn't need sT until
            # its group-A accumulation closes (~35us), so the slower scalar
            # ring is fine here.
            sT = [sT_p.tile([P, Bc], F32, tag=f"sT{J}", name=f"sT{J}")
                  for J in range(DT)]
            vrep = [sT_p.tile([P, P], BF16, tag=f"vrep{j}", name=f"vrep{j}")
                    for j in range(DT)]
            # -SMAX = -sum|v| >= -max-score bound: softmax is shift-invariant,
            # and exp(s - SMAX) in [e^-2|v|1, 1] stays comfortably in range,
            # so NO running max / rescale machinery is needed anywhere.
            nsm = sT_p.tile([P, 1], F32, tag="nsm", name="nsm")
            nc.scalar.dma_start(nsm[:], nsmax[:])
            with tc.tile_pool(name="setup_tmp", bufs=1) as tmp_p, \
                 tc.tile_pool(name="wsT", bufs=1) as wsT_p, \
                 tc.tile_pool(name="decT", bufs=1) as decT_p:
                dec_sb = tmp_p.tile([Bc, D], F32, tag="tmp4k", name="dec_sb")
                nc.scalar.dma_start(dec_sb[:], dec[:, :])
                decT = [decT_p.tile([P, Bc], BF16, tag=f"decT{a}", name=f"decT{a}")
                        for a in range(DT)]
                for a in range(DT):
                    ps = ps_t.tile([P, LCHUNK], F32, tag="ps_t")
                    nc.tensor.transpose(ps[:, :Bc], dec_sb[:, a * P:(a + 1) * P],
                                        ident[0:Bc, 0:Bc])
                    evac(decT[a][:], ps[:, :Bc])
                wst = [wsT_p.tile([P, D], BF16, tag=f"wsT{t}", name=f"wsT{t}")
                       for t in range(DT)]
                for t in range(DT):
                    nc.scalar.dma_start(wst[t][:], W_sT[t * P:(t + 1) * P, :])
                for j in range(DT):
                    nc.scalar.dma_start(vrep[j][:], v_rep[j])
                snat = tmp_p.tile([Bc, D], F32, tag="tmp4k", name="snat")
                SH = (D + 511) // 512
                for h in range(SH):
                    w = min(512, D - 512 * h)
                    ps_sh = ps_row.tile([Bc, 512], F32, tag="ps_row",
                                        name=f"ps_sh{h}")[:, :w]
                    for t in range(DT):
                        nc.tensor.matmul(ps_sh[:], decT[t][:],
                                         wst[t][:, 512 * h:512 * h + w],
                                         start=(t == 0), stop=(t == DT - 1))
                    evac(snat[:, 512 * h:512 * h + w], ps_sh[:])
                for J in range(DT):
                    ps = ps_t.tile([P, LCHUNK], F32, tag="ps_t")
                    nc.tensor.transpose(ps[:, :Bc], snat[:, J * P:(J + 1) * P],
                                        ident[0:Bc, 0:Bc])
                    evac(sT[J][:], ps[:, :Bc])

            def emit_chunk_mm_tmajor(b, ci, l0, lw, JG=4):
                """Batch-0 variant: t-major matmul groups so the PE is never
                head-of-line blocked on a late whT[t] while work on
                already-arrived tiles exists."""
                if ci == 0:
                    state[b]["erow"] = rows_p.tile(
                        [1, L], F32, tag="erow", name=f"erow{b}")
                encT = state[b][f"encT{ci}"]
                psc = ps_sc.tile([P, LCHUNK], F32, tag="ps_sc",
                                 name="psc")[:, :lw]
                for g in range(0, DT, JG):
                    phs = {j: ps_h.tile([P, LCHUNK], F32, tag="ps_h",
                                        name=f"ph{j}")
                           for j in range(g, g + JG)}
                    for t in range(ET):
                        for j in range(g, g + JG):
                            nc.tensor.matmul(phs[j][:, :lw],
                                             whT[t][:, j * P:(j + 1) * P],
                                             encT[:, t, :],
                                             start=(t == 0), stop=(t == ET - 1))
                    tjs = {}
                    for j in range(g, g + JG):
                        tj = t_p.tile([P, LCHUNK], BF16, tag="tj")
                        nc.scalar.activation(tj[:, :lw], phs[j][:, :lw],
                                             AF.Tanh, bias=sT[j][:, b:b + 1])
                        tjs[j] = tj
                    for j in range(g, g + JG):
                        nc.tensor.matmul(psc[:], vrep[j][:],
                                         tjs[j][:, :lw],
                                         start=(j == 0), stop=(j == DT - 1))
                state[b][f"psc{ci}"] = psc

            def emit_chunk_mm(b, ci, l0, lw):
                if ci == 0:
                    state[b]["erow"] = rows_p.tile(
                        [1, L], F32, tag="erow", name=f"erow{b}")
                encT = state[b][f"encT{ci}"]
                psc = ps_sc.tile([P, LCHUNK], F32, tag="ps_sc",
                                 name="psc")[:, :lw]
                tjs = {}
                for j in range(DT):
                    ph = ps_h.tile([P, LCHUNK], F32, tag="ps_h")
                    for t in range(ET):
                        nc.tensor.matmul(ph[:, :lw],
                                         whT[t][:, j * P:(j + 1) * P],
                                         encT[:, t, :],
                                         start=(t == 0), stop=(t == ET - 1))
                    tj = t_p.tile([P, LCHUNK], BF16, tag="tj")
                    nc.scalar.activation(tj[:, :lw], ph[:, :lw], AF.Tanh,
                                         bias=sT[j][:, b:b + 1])
                    tjs[j] = tj
                    # v-dot for d-block j-1 issues behind j's matmul group, so
                    # the PE never waits on the tanh chain of the same block.
                    if j > 0:
                        nc.tensor.matmul(psc[:], vrep[j - 1][:],
                                         tjs.pop(j - 1)[:, :lw],
                                         start=(j == 1), stop=False)
                nc.tensor.matmul(psc[:], vrep[DT - 1][:],
                                 tjs.pop(DT - 1)[:, :lw],
                                 start=False, stop=True)
                state[b][f"psc{ci}"] = psc

            def emit_flash(b, ci, l0, lw):
                """ctx accumulation for chunk ci of batch b, all
                [128, 1] / [128, ET] partition-parallel.  Thanks to the
                fixed -SMAX bias there is no running max: weights are
                exp(s - SMAX), summed and reduced directly."""
                psc = state[b].pop(f"psc{ci}")
                encT = state[b].pop(f"encT{ci}")
                par = ci % 2
                wcb = w_p.tile([P, lw], BF16, tag="wcb", name="wcb")
                zc = st_p.tile([P, 1], F32, tag=f"zc{par}", name=f"zc{par}")
                nc.scalar.activation(wcb[:], psc[:], AF.Exp, bias=nsm[:],
                                     accum_out=zc[:])
                # unnormalized attn row for this chunk (psc rows identical);
                # after wcb so it never delays the DVE ctx reduction below.
                nc.scalar.activation(state[b]["erow"][:, l0:l0 + lw],
                                     psc[0:1, :], AF.Exp, bias=nsm[0:1, :])
                # per-chunk ctx contribution: cchunk[:, t] = sum_l encT*wcb
                cchunk = st_p.tile([P, ET], F32, tag=f"cchunk{par}",
                                   name=f"cchunk{par}")
                scr = w_p.tile([P, LCHUNK], F32, tag="ttr_scr", name="ttr_scr")
                for t in range(ET):
                    nc.vector.scalar_tensor_tensor(
                        out=scr[:, :lw], in0=encT[:, t, :], scalar=1.0,
                        in1=wcb[:], op0=ALU.mult, op1=ALU.mult,
                        accum_out=cchunk[:, t:t + 1])
                if ci > 0:
                    z_old = state[b]["z"]
                    z = st_p.tile([P, 1], F32, tag=f"z{par}", name=f"z{par}")
                    nc.vector.tensor_add(out=z[:], in0=z_old[:], in1=zc[:])
                    ctx_old = state[b]["ctx_acc"]
                    ctx_acc = st_p.tile([P, ET], F32, tag=f"ctxn{par}",
                                        name=f"ctxn{par}")
                    nc.vector.tensor_add(out=ctx_acc[:], in0=ctx_old[:],
                                         in1=cchunk[:])
                else:
                    z = zc
                    ctx_acc = cchunk
                state[b]["ctx_acc"] = ctx_acc
                state[b]["z"] = z

            def emit_ctx_out(b):
                ctx_acc = state[b]["ctx_acc"]
                z = state[b]["z"]
                rz = st_p.tile([P, 1], F32, tag="rz128", name="rz128")
                nc.vector.reciprocal(rz[:], z[:])
                ps = ps_t.tile([P, LCHUNK], F32, tag="ps_t")
                nc.tensor.transpose(ps[0:ET, 0:P], ctx_acc[:], ident[:])
                cout = rows_p.tile([ET, P], F32, tag="cout", name=f"cout{b}")
                # rz rows are identical, so any slice of it is the right
                # per-partition scale in either layout.
                nc.scalar.activation(cout[:], ps[0:ET, 0:P], AF.Copy,
                                     scale=rz[0:ET, :])
                nc.sync.dma_start(ctx_o[b].rearrange("(a p) -> a p", a=ET),
                                  cout[:])
                erow = state[b]["erow"]
                arow = rows_p.tile([1, L], F32, tag="arow", name=f"arow{b}")
                nc.scalar.activation(arow[:], erow[:], AF.Copy,
                                     scale=rz[0:1, :])
                nc.sync.dma_start(attn_o[b:b + 1, :], arow[:])

            # Software pipeline: flash(ci) sits between load(ci+1) and
            # mm(ci+1); batch b+1's first chunk overlaps batch b's softmax
            # tail.  Batch 0 rides the startup DMA feed with t-major groups.
            CH0 = chunks_of(0)
            emit_chunk_mm_tmajor(0, 0, *CH0[0])
            for ci in range(1, len(CH0)):
                emit_chunk_load(0, ci, *CH0[ci])
                emit_flash(0, ci - 1, *CH0[ci - 1])
                emit_chunk_mm_tmajor(0, ci, *CH0[ci])
            for b in range(Bc):
                CHb = chunks_of(b)
                last = len(CHb) - 1
                if b + 1 < Bc:
                    CHn = chunks_of(b + 1)
                    emit_chunk_load(b + 1, 0, *CHn[0])
                    emit_flash(b, last, *CHb[last])
                    emit_chunk_mm(b + 1, 0, *CHn[0])
                    emit_ctx_out(b)
                    for ci in range(1, len(CHn)):
                        emit_chunk_load(b + 1, ci, *CHn[ci])
                        emit_flash(b + 1, ci - 1, *CHn[ci - 1])
                        emit_chunk_mm(b + 1, ci, *CHn[ci])
                else:
                    emit_flash(b, last, *CHb[last])
                    emit_ctx_out(b)

    return nc


_cache = {}


def _get_nc():
    if "nc" not in _cache:
        _install_waitsplit()
        _cache["nc"] = _build()
    return _cache["nc"]


def run(inputs, trace=False, **run_kwargs):
    """Run on 8 NeuronCores; returns (ctx, attn, BassKernelResults)."""
    import ml_dtypes
    from concourse.bass_utils import run_bass_kernel_spmd

    nc = _get_nc()
    bf16 = ml_dtypes.bfloat16
    enc = np.ascontiguousarray(
        np.asarray(inputs["enc_outputs"], dtype=np.float32).astype(bf16))
    dec = np.ascontiguousarray(np.asarray(inputs["dec_hidden"], dtype=np.float32))
    W_sT = np.ascontiguousarray(
        np.asarray(inputs["W_s"], dtype=np.float32).T.astype(bf16))
    W_hT = np.ascontiguousarray(
        np.asarray(inputs["W_h"], dtype=np.float32).T.astype(bf16))
    v = np.asarray(inputs["v"], dtype=np.float32)
    v_rep = np.ascontiguousarray(
        np.broadcast_to(v.reshape(_D // 128, 128, 1),
                        (_D // 128, 128, 128)).astype(bf16))
    nsmax = np.full((128, 1), -np.abs(v).sum(), dtype=np.float32)
    B = enc.shape[0]
    Bc = B // _NCORES
    in_maps = []
    for i in range(_NCORES):
        in_maps.append({
            "enc": enc[i * Bc:(i + 1) * Bc],
            "dec": dec[i * Bc:(i + 1) * Bc],
            "W_sT": W_sT,
            "W_hT": W_hT,
            "v_rep": v_rep,
            "nsmax": nsmax,
        })
    res = run_bass_kernel_spmd(nc, in_maps, core_ids=list(range(_NCORES)),
                               trace=trace, **run_kwargs)
    ctx = np.concatenate([res.results[i]["ctx"] for i in range(_NCORES)], axis=0)
    attn = np.concatenate([res.results[i]["attn"] for i in range(_NCORES)], axis=0)
    return ctx, attn, res


def kernel(**inputs):
    ctx, attn, _ = run(inputs, trace=False)
    return ctx, attn
